# revision 13
# baseline (speedup 1.0000x reference)
"""DenseGCM kernel for 8 Trainium2 NeuronCores.

Reference semantics (per batch b of 64):
  1. wrap-eviction if num_nodes[b] == 1024 (zero slot 0, roll up by one)
  2. scatter x[b] into nodes[b, idx] (idx = post-wrap num_nodes)
  3. node_feats = tanh((adj @ nodes) @ W); mx = node_feats[idx]
  4. return (mx, nodes, adj, num_nodes + 1)

node_feats is only read back at row idx, so the GNN collapses to a
matvec chain: mx[b] = tanh((adj_w[b, idx] @ nodes_w[b]) @ W).  The
dominant cost is pure memory movement of adj (4MB/batch in + out).

Sharding: pure data parallel, 8 batches per core.  Per core:
  - adj copied DRAM->DRAM (the wrap shift for the ~1/1025-rare wrapped
    batches is applied on host when building the shard).
  - nodes streamed DRAM->SBUF->DRAM; while each [1024, 256] batch tile
    is resident, TensorE computes yT = nodes_w^T @ adj_row (vertical
    [256, 1]), then z = yT^T @ W, then ScalarE tanh -> mx row.
  - x rows scattered into nodes_out by one indirect DMA (indices are
    host-computed from num_nodes, which is an input and host-visible).
  - the scatter also perturbs y; the rank-1 fixup
    ycorr = adj_row[idx] * (x - nodes_w[idx]) is a host-computed
    256-vector added to yT on device.
"""

import numpy as np

B, N, F = 64, 1024, 256
NCORES = 8
SB = B // NCORES  # batches per core
NT = N // 128     # m-tiles per batch
FT = F // 128     # feature half-tiles

_CACHE = {}
TRACE = False  # dev harness hook: capture NTFF profile via run_bass_kernel_spmd


def _build(loop=1):
    import concourse.mybir as mybir
    import concourse.tile as tile
    from concourse import bacc
    from concourse.bass import IndirectOffsetOnAxis
    from concourse.tile import add_dep_helper

    f32 = mybir.dt.float32
    i32 = mybir.dt.int32

    nc = bacc.Bacc("TRN2", target_bir_lowering=False, debug=False)

    nodes_in = nc.dram_tensor("nodes_in", [SB * N, F], f32, kind="ExternalInput")
    adj_in = nc.dram_tensor("adj_in", [SB, N, N], f32, kind="ExternalInput")
    x_in = nc.dram_tensor("x_in", [SB, F], f32, kind="ExternalInput")
    w_in = nc.dram_tensor("w_in", [F, F], f32, kind="ExternalInput")
    rowsv_in = nc.dram_tensor("rowsv_in", [128, SB * NT], f32, kind="ExternalInput")
    ycorrv_in = nc.dram_tensor("ycorrv_in", [128, SB * FT], f32, kind="ExternalInput")
    sidx_in = nc.dram_tensor("sidx_in", [SB, 1], i32, kind="ExternalInput")

    nodes_out = nc.dram_tensor("nodes_out", [SB * N, F], f32, kind="ExternalOutput")
    adj_out = nc.dram_tensor("adj_out", [SB, N, N], f32, kind="ExternalOutput")
    mx_out = nc.dram_tensor("mx_out", [SB, F], f32, kind="ExternalOutput")

    with tile.TileContext(nc) as tc:
        with (
            tc.tile_pool(name="const", bufs=1) as const_pool,
            tc.tile_pool(name="nodes", bufs=3) as node_pool,
            tc.tile_pool(name="small", bufs=2) as small_pool,
            tc.tile_pool(name="mx", bufs=2) as mx_pool,
            tc.tile_pool(name="yps", bufs=2, space="PSUM") as ypool,
            tc.tile_pool(name="zps", bufs=2, space="PSUM") as zpool,
        ):
            wv = const_pool.tile([128, FT, F], f32)
            nc.scalar.dma_start(wv[:], w_in[:].rearrange("(k p) g -> p k g", p=128))
            rowsv = const_pool.tile([128, SB * NT], f32)
            nc.scalar.dma_start(rowsv[:], rowsv_in[:])
            ycorrv = const_pool.tile([128, SB * FT], f32)
            nc.scalar.dma_start(ycorrv[:], ycorrv_in[:])
            xrow = const_pool.tile([SB, F], f32)
            nc.scalar.dma_start(xrow[:], x_in[:])
            sidx = const_pool.tile([SB, 1], i32)
            nc.scalar.dma_start(sidx[:], sidx_in[:])

            for it in range(loop):
                # adj passthrough: DRAM->DRAM, one 4MB chunk per batch on
                # the SP HWDGE ring so it streams concurrently with the
                # nodes traffic on the ACT ring.
                for b in range(SB):
                    nc.sync.dma_start(adj_out[b, :, :], adj_in[b, :, :])

                store_insts = []
                for b in range(SB):
                    nt = node_pool.tile([128, NT, F], f32)
                    src = nodes_in[b * N : (b + 1) * N, :].rearrange(
                        "(t p) f -> p t f", p=128
                    )
                    nc.scalar.dma_start(nt[:], src)

                    # yT[fh*128+p] = sum_m nodes_w[m, f] * adj_row[m]
                    yps = ypool.tile([128, FT], f32)
                    for fh in range(FT):
                        for t in range(NT):
                            nc.tensor.matmul(
                                yps[:, fh : fh + 1],
                                lhsT=nt[:, t, fh * 128 : (fh + 1) * 128],
                                rhs=rowsv[:, b * NT + t : b * NT + t + 1],
                                start=(t == 0),
                                stop=(t == NT - 1),
                            )
                    yt = small_pool.tile([128, FT], f32)
                    nc.vector.tensor_add(
                        yt[:], yps[:], ycorrv[:, b * FT : (b + 1) * FT]
                    )

                    # z[g] = sum_f yT[f] * W[f, g]
                    zps = zpool.tile([1, F], f32)
                    for kt in range(FT):
                        nc.tensor.matmul(
                            zps[:1, :],
                            lhsT=yt[:, kt : kt + 1],
                            rhs=wv[:, kt, :],
                            start=(kt == 0),
                            stop=(kt == FT - 1),
                        )
                    mxs = mx_pool.tile([1, F], f32)
                    nc.scalar.activation(
                        mxs[:1, :], zps[:1, :], mybir.ActivationFunctionType.Tanh
                    )
                    nc.scalar.dma_start(mx_out[b : b + 1, :], mxs[:1, :])

                    dst = nodes_out[b * N : (b + 1) * N, :].rearrange(
                        "(t p) f -> p t f", p=128
                    )
                    store_insts.append(nc.scalar.dma_start(dst, nt[:]))

                # scatter x rows into nodes_out after the bulk stores
                scat = nc.gpsimd.indirect_dma_start(
                    out=nodes_out[:],
                    out_offset=IndirectOffsetOnAxis(ap=sidx[:, :1], axis=0),
                    in_=xrow[:],
                    in_offset=None,
                )
                for st in store_insts:
                    add_dep_helper(
                        scat.ins, st.ins, reason="scatter after bulk node stores"
                    )

    nc.compile()
    return nc


def _get_nc():
    if "nc" not in _CACHE:
        _CACHE["nc"] = _build()
    return _CACHE["nc"]


def _prepare(x, nodes, adj, num_nodes, W):
    """Host-side prep: shard, pre-apply the rare wrap shift, build aux
    index/correction tensors.  Returns (in_maps, num_out)."""
    x = np.asarray(x)
    nodes = np.asarray(nodes)
    adj = np.asarray(adj)
    num_nodes = np.asarray(num_nodes)
    W = np.asarray(W)

    wrap = (num_nodes.astype(np.int64) + 1) > N
    idx = np.where(wrap, num_nodes - 1, num_nodes).astype(np.int64)  # post-wrap slot
    num_out = (idx + 1).astype(num_nodes.dtype)

    x32 = np.ascontiguousarray(x, dtype=np.float32)
    W32 = np.ascontiguousarray(W, dtype=np.float32)

    in_maps = []
    for c in range(NCORES):
        s = c * SB
        nodes_s = nodes[s : s + SB]
        adj_s = adj[s : s + SB]
        wl = np.nonzero(wrap[s : s + SB])[0]
        if wl.size:
            nodes_s = nodes_s.copy()
            adj_s = adj_s.copy()
            for b in wl:
                nodes_s[b, :-1] = nodes[s + b, 1:]
                nodes_s[b, -1] = 0.0
                adj_s[b, :-1, :-1] = adj[s + b, 1:, 1:]
                adj_s[b, -1, :] = 0.0
                adj_s[b, :, -1] = 0.0
        nodes_s = np.ascontiguousarray(nodes_s, dtype=np.float32)
        adj_s = np.ascontiguousarray(adj_s, dtype=np.float32)

        li = idx[s : s + SB]
        # adj_w row of the written slot, vertical layout rowsv[p, b*NT+t]
        rows = adj_s[np.arange(SB), li]              # [SB, N]
        rowsv = (
            rows.reshape(SB, NT, 128).transpose(2, 0, 1).reshape(128, SB * NT)
        )
        # rank-1 scatter fixup: ycorr[b] = adj_row[idx]*(x - nodes_w[idx])
        diag = rows[np.arange(SB), li]               # [SB]
        nrow = nodes_s[np.arange(SB), li]            # [SB, F]
        ycorr = diag[:, None] * (x32[s : s + SB] - nrow)
        ycorrv = (
            ycorr.reshape(SB, FT, 128).transpose(2, 0, 1).reshape(128, SB * FT)
        )
        sidx = (np.arange(SB) * N + li).astype(np.int32).reshape(SB, 1)

        in_maps.append(
            {
                "nodes_in": nodes_s.reshape(SB * N, F),
                "adj_in": adj_s,
                "x_in": x32[s : s + SB],
                "w_in": W32,
                "rowsv_in": np.ascontiguousarray(rowsv, dtype=np.float32),
                "ycorrv_in": np.ascontiguousarray(ycorrv, dtype=np.float32),
                "sidx_in": sidx,
            }
        )

    return in_maps, num_out


def _get_runner():
    """Persistent jitted shard_map callable over all 8 cores (mirrors
    run_bass_via_pjrt, but built once and reused across kernel() calls)."""
    if "runner" in _CACHE:
        return _CACHE["runner"]

    import jax
    import jax.numpy as jnp
    from jax.sharding import Mesh, NamedSharding, PartitionSpec

    import warnings

    try:
        with warnings.catch_warnings():
            warnings.simplefilter("ignore")
            from jax.experimental.shard_map import shard_map as _sm

        def shard_map(f, mesh, in_specs, out_specs, check_rep):
            return _sm(
                f, mesh=mesh, in_specs=in_specs, out_specs=out_specs,
                check_rep=check_rep,
            )
    except ImportError:
        from jax import shard_map as _sm

        def shard_map(f, mesh, in_specs, out_specs, check_rep):
            return _sm(f, mesh=mesh, in_specs=in_specs, out_specs=out_specs)

    import concourse.mybir as mybir
    from concourse import bass2jax
    from concourse.bass2jax import _bass_exec_p, install_neuronx_cc_hook

    nc = _get_nc()
    install_neuronx_cc_hook()
    partition_name = nc.partition_id_tensor.name if nc.partition_id_tensor else None
    in_names, out_names, out_avals = [], [], []
    for alloc in nc.m.functions[0].allocations:
        if not isinstance(alloc, mybir.MemoryLocationSet):
            continue
        name = alloc.memorylocations[0].name
        if alloc.kind == "ExternalInput":
            if name != partition_name:
                in_names.append(name)
        elif alloc.kind == "ExternalOutput":
            out_names.append(name)
            out_avals.append(
                jax.core.ShapedArray(
                    tuple(alloc.tensor_shape), mybir.dt.np(alloc.dtype)
                )
            )
    n_params = len(in_names)
    bind_names = tuple(
        in_names + out_names + ([partition_name] if partition_name else [])
    )

    def _body(*args):
        operands = list(args)
        if partition_name is not None:
            operands.append(bass2jax.partition_id_tensor())
        return tuple(
            _bass_exec_p.bind(
                *operands,
                out_avals=tuple(out_avals),
                in_names=bind_names,
                out_names=tuple(out_names),
                lowering_input_output_aliases=(),
                sim_require_finite=True,
                sim_require_nnan=True,
                nc=nc,
            )
        )

    devices = jax.devices()[:NCORES]
    mesh = Mesh(np.asarray(devices), ("core",))
    nin = n_params + len(out_names)
    donate = tuple(range(n_params, nin))
    sharded = jax.jit(
        shard_map(
            _body,
            mesh=mesh,
            in_specs=(PartitionSpec("core"),) * nin,
            out_specs=(PartitionSpec("core"),) * len(out_names),
            check_rep=False,
        ),
        donate_argnums=donate,
    )
    shard = NamedSharding(mesh, PartitionSpec("core"))

    def run(in_maps):
        dev_in = [
            jax.device_put(
                np.concatenate([np.asarray(m[nm]) for m in in_maps], axis=0),
                shard,
            )
            for nm in in_names
        ]
        zeros = [
            jax.device_put(
                jnp.zeros((NCORES * a.shape[0], *a.shape[1:]), a.dtype), shard
            )
            for a in out_avals
        ]
        outs = sharded(*dev_in, *zeros)
        return {nm: np.asarray(o) for nm, o in zip(out_names, outs)}

    _CACHE["runner"] = run
    return run


def kernel(x, nodes, adj, num_nodes, W):
    in_maps, num_out = _prepare(x, nodes, adj, num_nodes, W)

    try:
        out = _get_runner()(in_maps)
        mx = out["mx_out"]
        nodes_full = out["nodes_out"].reshape(B, N, F)
        adj_full = out["adj_out"].reshape(B, N, N)
    except Exception:
        # robust fallback: the stock SPMD path
        from concourse.bass_utils import run_bass_kernel_spmd

        _CACHE.pop("runner", None)
        res = run_bass_kernel_spmd(
            _get_nc(), in_maps, list(range(NCORES)), trace=TRACE
        )
        _CACHE["last_res"] = res
        mx = np.concatenate([r["mx_out"] for r in res.results], axis=0)
        nodes_full = np.concatenate(
            [r["nodes_out"].reshape(SB, N, F) for r in res.results], axis=0
        )
        adj_full = np.concatenate([r["adj_out"] for r in res.results], axis=0)
    return mx, nodes_full, adj_full, num_out


# revision 14
# speedup vs baseline: 2.5299x; 2.5299x over previous
"""DenseGCM kernel for 8 Trainium2 NeuronCores.

Reference semantics (per batch b of 64):
  1. wrap-eviction if num_nodes[b] == 1024 (zero slot 0, roll up by one)
  2. scatter x[b] into nodes[b, idx] (idx = post-wrap num_nodes)
  3. node_feats = tanh((adj @ nodes) @ W); mx = node_feats[idx]
  4. return (mx, nodes, adj, num_nodes + 1)

node_feats is only read back at row idx, so the GNN collapses to a
matvec chain: mx[b] = tanh((adj_w[b, idx] @ nodes_w[b]) @ W).  The
dominant cost is pure memory movement of adj (4MB/batch in + out).

Sharding: pure data parallel, 8 batches per core.  Per core:
  - adj copied DRAM->DRAM (the wrap shift for the ~1/1025-rare wrapped
    batches is applied on host when building the shard).
  - nodes streamed DRAM->SBUF->DRAM; while each [1024, 256] batch tile
    is resident, TensorE computes yT = nodes_w^T @ adj_row (vertical
    [256, 1]), then z = yT^T @ W, then ScalarE tanh -> mx row.
  - x rows scattered into nodes_out by one indirect DMA (indices are
    host-computed from num_nodes, which is an input and host-visible).
  - the scatter also perturbs y; the rank-1 fixup
    ycorr = adj_row[idx] * (x - nodes_w[idx]) is a host-computed
    256-vector added to yT on device.
"""

import numpy as np

B, N, F = 64, 1024, 256
NCORES = 8
SB = B // NCORES  # batches per core
NT = N // 128     # m-tiles per batch
FT = F // 128     # feature half-tiles

_CACHE = {}
TRACE = False  # dev harness hook: capture NTFF profile via run_bass_kernel_spmd


def _build(loop=1):
    import concourse.mybir as mybir
    import concourse.tile as tile
    from concourse import bacc
    from concourse.bass import IndirectOffsetOnAxis
    from concourse.tile import add_dep_helper

    f32 = mybir.dt.float32
    i32 = mybir.dt.int32

    nc = bacc.Bacc("TRN2", target_bir_lowering=False, debug=False)

    nodes_in = nc.dram_tensor("nodes_in", [SB * N, F], f32, kind="ExternalInput")
    adj_in = nc.dram_tensor("adj_in", [SB, N, N], f32, kind="ExternalInput")
    x_in = nc.dram_tensor("x_in", [SB, F], f32, kind="ExternalInput")
    w_in = nc.dram_tensor("w_in", [F, F], f32, kind="ExternalInput")
    rowsv_in = nc.dram_tensor("rowsv_in", [128, SB * NT], f32, kind="ExternalInput")
    ycorrv_in = nc.dram_tensor("ycorrv_in", [128, SB * FT], f32, kind="ExternalInput")
    sidx_in = nc.dram_tensor("sidx_in", [SB, 1], i32, kind="ExternalInput")

    nodes_out = nc.dram_tensor("nodes_out", [SB * N, F], f32, kind="ExternalOutput")
    adj_out = nc.dram_tensor("adj_out", [SB, N, N], f32, kind="ExternalOutput")
    mx_out = nc.dram_tensor("mx_out", [SB, F], f32, kind="ExternalOutput")

    with tile.TileContext(nc) as tc:
        with (
            tc.tile_pool(name="const", bufs=1) as const_pool,
            tc.tile_pool(name="nodes", bufs=3) as node_pool,
            tc.tile_pool(name="small", bufs=2) as small_pool,
            tc.tile_pool(name="mx", bufs=2) as mx_pool,
            tc.tile_pool(name="yps", bufs=2, space="PSUM") as ypool,
            tc.tile_pool(name="zps", bufs=2, space="PSUM") as zpool,
        ):
            wv = const_pool.tile([128, FT, F], f32)
            nc.scalar.dma_start(wv[:], w_in[:].rearrange("(k p) g -> p k g", p=128))
            rowsv = const_pool.tile([128, SB * NT], f32)
            nc.scalar.dma_start(rowsv[:], rowsv_in[:])
            ycorrv = const_pool.tile([128, SB * FT], f32)
            nc.scalar.dma_start(ycorrv[:], ycorrv_in[:])
            xrow = const_pool.tile([SB, F], f32)
            nc.scalar.dma_start(xrow[:], x_in[:])
            sidx = const_pool.tile([SB, 1], i32)
            nc.scalar.dma_start(sidx[:], sidx_in[:])

            for it in range(loop):
                # adj passthrough: DRAM->DRAM, one 4MB chunk per batch on
                # the SP HWDGE ring so it streams concurrently with the
                # nodes traffic on the ACT ring.
                for b in range(SB):
                    nc.sync.dma_start(adj_out[b, :, :], adj_in[b, :, :])

                store_insts = []
                for b in range(SB):
                    nt = node_pool.tile([128, NT, F], f32)
                    src = nodes_in[b * N : (b + 1) * N, :].rearrange(
                        "(p t) f -> p t f", p=128
                    )
                    nc.scalar.dma_start(nt[:], src)

                    # yT[fh*128+p] = sum_m nodes_w[m, f] * adj_row[m]
                    yps = ypool.tile([128, FT], f32)
                    for fh in range(FT):
                        for t in range(NT):
                            nc.tensor.matmul(
                                yps[:, fh : fh + 1],
                                lhsT=nt[:, t, fh * 128 : (fh + 1) * 128],
                                rhs=rowsv[:, b * NT + t : b * NT + t + 1],
                                start=(t == 0),
                                stop=(t == NT - 1),
                            )
                    yt = small_pool.tile([128, FT], f32)
                    nc.vector.tensor_add(
                        yt[:], yps[:], ycorrv[:, b * FT : (b + 1) * FT]
                    )

                    # z[g] = sum_f yT[f] * W[f, g]
                    zps = zpool.tile([1, F], f32)
                    for kt in range(FT):
                        nc.tensor.matmul(
                            zps[:1, :],
                            lhsT=yt[:, kt : kt + 1],
                            rhs=wv[:, kt, :],
                            start=(kt == 0),
                            stop=(kt == FT - 1),
                        )
                    mxs = mx_pool.tile([1, F], f32)
                    nc.scalar.activation(
                        mxs[:1, :], zps[:1, :], mybir.ActivationFunctionType.Tanh
                    )
                    nc.scalar.dma_start(mx_out[b : b + 1, :], mxs[:1, :])

                    dst = nodes_out[b * N : (b + 1) * N, :].rearrange(
                        "(p t) f -> p t f", p=128
                    )
                    store_insts.append(nc.scalar.dma_start(dst, nt[:]))

                # scatter x rows into nodes_out after the bulk stores
                scat = nc.gpsimd.indirect_dma_start(
                    out=nodes_out[:],
                    out_offset=IndirectOffsetOnAxis(ap=sidx[:, :1], axis=0),
                    in_=xrow[:],
                    in_offset=None,
                )
                for st in store_insts:
                    add_dep_helper(
                        scat.ins, st.ins, reason="scatter after bulk node stores"
                    )

    nc.compile()
    return nc


def _get_nc():
    if "nc" not in _CACHE:
        _CACHE["nc"] = _build()
    return _CACHE["nc"]


def _prepare(x, nodes, adj, num_nodes, W):
    """Host-side prep: shard, pre-apply the rare wrap shift, build aux
    index/correction tensors.  Returns (in_maps, num_out)."""
    x = np.asarray(x)
    nodes = np.asarray(nodes)
    adj = np.asarray(adj)
    num_nodes = np.asarray(num_nodes)
    W = np.asarray(W)

    wrap = (num_nodes.astype(np.int64) + 1) > N
    idx = np.where(wrap, num_nodes - 1, num_nodes).astype(np.int64)  # post-wrap slot
    num_out = (idx + 1).astype(num_nodes.dtype)

    x32 = np.ascontiguousarray(x, dtype=np.float32)
    W32 = np.ascontiguousarray(W, dtype=np.float32)

    in_maps = []
    for c in range(NCORES):
        s = c * SB
        nodes_s = nodes[s : s + SB]
        adj_s = adj[s : s + SB]
        wl = np.nonzero(wrap[s : s + SB])[0]
        if wl.size:
            nodes_s = nodes_s.copy()
            adj_s = adj_s.copy()
            for b in wl:
                nodes_s[b, :-1] = nodes[s + b, 1:]
                nodes_s[b, -1] = 0.0
                adj_s[b, :-1, :-1] = adj[s + b, 1:, 1:]
                adj_s[b, -1, :] = 0.0
                adj_s[b, :, -1] = 0.0
        nodes_s = np.ascontiguousarray(nodes_s, dtype=np.float32)
        adj_s = np.ascontiguousarray(adj_s, dtype=np.float32)

        li = idx[s : s + SB]
        # adj_w row of the written slot, vertical layout rowsv[p, b*NT+t]
        rows = adj_s[np.arange(SB), li]              # [SB, N]
        rowsv = (
            rows.reshape(SB, 128, NT).transpose(1, 0, 2).reshape(128, SB * NT)
        )
        # rank-1 scatter fixup: ycorr[b] = adj_row[idx]*(x - nodes_w[idx])
        diag = rows[np.arange(SB), li]               # [SB]
        nrow = nodes_s[np.arange(SB), li]            # [SB, F]
        ycorr = diag[:, None] * (x32[s : s + SB] - nrow)
        ycorrv = (
            ycorr.reshape(SB, FT, 128).transpose(2, 0, 1).reshape(128, SB * FT)
        )
        sidx = (np.arange(SB) * N + li).astype(np.int32).reshape(SB, 1)

        in_maps.append(
            {
                "nodes_in": nodes_s.reshape(SB * N, F),
                "adj_in": adj_s,
                "x_in": x32[s : s + SB],
                "w_in": W32,
                "rowsv_in": np.ascontiguousarray(rowsv, dtype=np.float32),
                "ycorrv_in": np.ascontiguousarray(ycorrv, dtype=np.float32),
                "sidx_in": sidx,
            }
        )

    return in_maps, num_out


def _get_runner():
    """Persistent jitted shard_map callable over all 8 cores (mirrors
    run_bass_via_pjrt, but built once and reused across kernel() calls)."""
    if "runner" in _CACHE:
        return _CACHE["runner"]

    import jax
    import jax.numpy as jnp
    from jax.sharding import Mesh, NamedSharding, PartitionSpec

    import warnings

    try:
        with warnings.catch_warnings():
            warnings.simplefilter("ignore")
            from jax.experimental.shard_map import shard_map as _sm

        def shard_map(f, mesh, in_specs, out_specs, check_rep):
            return _sm(
                f, mesh=mesh, in_specs=in_specs, out_specs=out_specs,
                check_rep=check_rep,
            )
    except ImportError:
        from jax import shard_map as _sm

        def shard_map(f, mesh, in_specs, out_specs, check_rep):
            return _sm(f, mesh=mesh, in_specs=in_specs, out_specs=out_specs)

    import concourse.mybir as mybir
    from concourse import bass2jax
    from concourse.bass2jax import _bass_exec_p, install_neuronx_cc_hook

    nc = _get_nc()
    install_neuronx_cc_hook()
    partition_name = nc.partition_id_tensor.name if nc.partition_id_tensor else None
    in_names, out_names, out_avals = [], [], []
    for alloc in nc.m.functions[0].allocations:
        if not isinstance(alloc, mybir.MemoryLocationSet):
            continue
        name = alloc.memorylocations[0].name
        if alloc.kind == "ExternalInput":
            if name != partition_name:
                in_names.append(name)
        elif alloc.kind == "ExternalOutput":
            out_names.append(name)
            out_avals.append(
                jax.core.ShapedArray(
                    tuple(alloc.tensor_shape), mybir.dt.np(alloc.dtype)
                )
            )
    n_params = len(in_names)
    bind_names = tuple(
        in_names + out_names + ([partition_name] if partition_name else [])
    )

    def _body(*args):
        operands = list(args)
        if partition_name is not None:
            operands.append(bass2jax.partition_id_tensor())
        return tuple(
            _bass_exec_p.bind(
                *operands,
                out_avals=tuple(out_avals),
                in_names=bind_names,
                out_names=tuple(out_names),
                lowering_input_output_aliases=(),
                sim_require_finite=True,
                sim_require_nnan=True,
                nc=nc,
            )
        )

    devices = jax.devices()[:NCORES]
    mesh = Mesh(np.asarray(devices), ("core",))
    nin = n_params + len(out_names)
    donate = tuple(range(n_params, nin))
    sharded = jax.jit(
        shard_map(
            _body,
            mesh=mesh,
            in_specs=(PartitionSpec("core"),) * nin,
            out_specs=(PartitionSpec("core"),) * len(out_names),
            check_rep=False,
        ),
        donate_argnums=donate,
    )
    shard = NamedSharding(mesh, PartitionSpec("core"))

    def run(in_maps):
        dev_in = [
            jax.device_put(
                np.concatenate([np.asarray(m[nm]) for m in in_maps], axis=0),
                shard,
            )
            for nm in in_names
        ]
        zeros = [
            jax.device_put(
                jnp.zeros((NCORES * a.shape[0], *a.shape[1:]), a.dtype), shard
            )
            for a in out_avals
        ]
        outs = sharded(*dev_in, *zeros)
        return {nm: np.asarray(o) for nm, o in zip(out_names, outs)}

    _CACHE["runner"] = run
    return run


def kernel(x, nodes, adj, num_nodes, W):
    in_maps, num_out = _prepare(x, nodes, adj, num_nodes, W)

    try:
        out = _get_runner()(in_maps)
        mx = out["mx_out"]
        nodes_full = out["nodes_out"].reshape(B, N, F)
        adj_full = out["adj_out"].reshape(B, N, N)
    except Exception:
        # robust fallback: the stock SPMD path
        from concourse.bass_utils import run_bass_kernel_spmd

        _CACHE.pop("runner", None)
        res = run_bass_kernel_spmd(
            _get_nc(), in_maps, list(range(NCORES)), trace=TRACE
        )
        _CACHE["last_res"] = res
        mx = np.concatenate([r["mx_out"] for r in res.results], axis=0)
        nodes_full = np.concatenate(
            [r["nodes_out"].reshape(SB, N, F) for r in res.results], axis=0
        )
        adj_full = np.concatenate([r["adj_out"] for r in res.results], axis=0)
    return mx, nodes_full, adj_full, num_out


# revision 17
# speedup vs baseline: 2.7870x; 1.1016x over previous
"""DenseGCM kernel for 8 Trainium2 NeuronCores.

Reference semantics (per batch b of 64):
  1. wrap-eviction if num_nodes[b] == 1024 (zero slot 0, roll up by one)
  2. scatter x[b] into nodes[b, idx] (idx = post-wrap num_nodes)
  3. node_feats = tanh((adj @ nodes) @ W); mx = node_feats[idx]
  4. return (mx, nodes, adj, num_nodes + 1)

node_feats is only read back at row idx, so the GNN collapses to a
matvec chain: mx[b] = tanh((adj_w[b, idx] @ nodes_w[b]) @ W).  The
dominant cost is pure memory movement of adj (4MB/batch in + out).

Sharding: pure data parallel, 8 batches per core.  Per core:
  - adj copied DRAM->DRAM (the wrap shift for the ~1/1025-rare wrapped
    batches is applied on host when building the shard).
  - nodes streamed DRAM->SBUF->DRAM; while each [1024, 256] batch tile
    is resident, TensorE computes yT = nodes_w^T @ adj_row (vertical
    [256, 1]), then z = yT^T @ W, then ScalarE tanh -> mx row.
  - x rows scattered into nodes_out by one indirect DMA (indices are
    host-computed from num_nodes, which is an input and host-visible).
  - the scatter also perturbs y; the rank-1 fixup
    ycorr = adj_row[idx] * (x - nodes_w[idx]) is a host-computed
    256-vector added to yT on device.

Node index m maps to (partition, slot) as m = p*8 + t so each DMA
descriptor moves a contiguous 2KB run per partition.
"""

import numpy as np

B, N, F = 64, 1024, 256
NCORES = 8
SB = B // NCORES  # batches per core
NT = N // 128     # m-slots per partition
FT = F // 128     # feature half-tiles

_CACHE = {}
TRACE = False  # dev harness hook (run_bass_kernel_spmd fallback path only)


def _build(loop=1, hw_loop=False):
    import contextlib

    import concourse.mybir as mybir
    import concourse.tile as tile
    from concourse import bacc
    from concourse.bass import IndirectOffsetOnAxis
    from concourse.tile import add_dep_helper

    f32 = mybir.dt.float32
    i32 = mybir.dt.int32

    nc = bacc.Bacc("TRN2", target_bir_lowering=False, debug=False)

    nodes_in = nc.dram_tensor("nodes_in", [SB * N, F], f32, kind="ExternalInput")
    adj_in = nc.dram_tensor("adj_in", [SB, N, N], f32, kind="ExternalInput")
    x_in = nc.dram_tensor("x_in", [SB, F], f32, kind="ExternalInput")
    w_in = nc.dram_tensor("w_in", [F, F], f32, kind="ExternalInput")
    rowsv_in = nc.dram_tensor("rowsv_in", [128, SB * NT], f32, kind="ExternalInput")
    ycorrv_in = nc.dram_tensor("ycorrv_in", [128, SB * FT], f32, kind="ExternalInput")
    sidx_in = nc.dram_tensor("sidx_in", [SB, 1], i32, kind="ExternalInput")

    nodes_out = nc.dram_tensor("nodes_out", [SB * N, F], f32, kind="ExternalOutput")
    adj_out = nc.dram_tensor("adj_out", [SB, N, N], f32, kind="ExternalOutput")
    mx_out = nc.dram_tensor("mx_out", [SB, F], f32, kind="ExternalOutput")

    with tile.TileContext(nc) as tc:
        with (
            tc.tile_pool(name="const", bufs=1) as const_pool,
            tc.tile_pool(name="nodes", bufs=3) as node_pool,
            tc.tile_pool(name="small", bufs=2) as small_pool,
            tc.tile_pool(name="mx", bufs=2) as mx_pool,
            tc.tile_pool(name="yps", bufs=2, space="PSUM") as ypool,
            tc.tile_pool(name="zps", bufs=2, space="PSUM") as zpool,
        ):
            wv = const_pool.tile([128, FT, F], f32)
            nc.scalar.dma_start(wv[:], w_in[:].rearrange("(k p) g -> p k g", p=128))
            rowsv = const_pool.tile([128, SB * NT], f32)
            nc.scalar.dma_start(rowsv[:], rowsv_in[:])
            ycorrv = const_pool.tile([128, SB * FT], f32)
            nc.scalar.dma_start(ycorrv[:], ycorrv_in[:])
            xrow = const_pool.tile([SB, F], f32)
            nc.scalar.dma_start(xrow[:], x_in[:])
            sidx = const_pool.tile([SB, 1], i32)
            nc.scalar.dma_start(sidx[:], sidx_in[:])

            def emit_body():
                # adj passthrough: DRAM->DRAM, one 4MB chunk per batch on
                # the SP HWDGE ring so it streams concurrently with the
                # nodes traffic on the ACT ring.
                for b in range(SB):
                    nc.sync.dma_start(adj_out[b, :, :], adj_in[b, :, :])

                store_insts = []
                for b in range(SB):
                    nt = node_pool.tile([128, NT, F], f32)
                    src = nodes_in[b * N : (b + 1) * N, :].rearrange(
                        "(p t) f -> p t f", p=128
                    )
                    nc.scalar.dma_start(nt[:], src)

                    # yT[fh*128+p] = sum_m nodes_w[m, f] * adj_row[m]
                    yps = ypool.tile([128, FT], f32)
                    for fh in range(FT):
                        for t in range(NT):
                            nc.tensor.matmul(
                                yps[:, fh : fh + 1],
                                lhsT=nt[:, t, fh * 128 : (fh + 1) * 128],
                                rhs=rowsv[:, b * NT + t : b * NT + t + 1],
                                start=(t == 0),
                                stop=(t == NT - 1),
                            )
                    yt = small_pool.tile([128, FT], f32)
                    nc.vector.tensor_add(
                        yt[:], yps[:], ycorrv[:, b * FT : (b + 1) * FT]
                    )

                    # z[g] = sum_f yT[f] * W[f, g]
                    zps = zpool.tile([1, F], f32)
                    for kt in range(FT):
                        nc.tensor.matmul(
                            zps[:1, :],
                            lhsT=yt[:, kt : kt + 1],
                            rhs=wv[:, kt, :],
                            start=(kt == 0),
                            stop=(kt == FT - 1),
                        )
                    mxs = mx_pool.tile([1, F], f32)
                    nc.scalar.activation(
                        mxs[:1, :], zps[:1, :], mybir.ActivationFunctionType.Tanh
                    )
                    nc.scalar.dma_start(mx_out[b : b + 1, :], mxs[:1, :])

                    dst = nodes_out[b * N : (b + 1) * N, :].rearrange(
                        "(p t) f -> p t f", p=128
                    )
                    store_insts.append(nc.scalar.dma_start(dst, nt[:]))

                # scatter x rows into nodes_out after the bulk stores
                scat = nc.gpsimd.indirect_dma_start(
                    out=nodes_out[:],
                    out_offset=IndirectOffsetOnAxis(ap=sidx[:, :1], axis=0),
                    in_=xrow[:],
                    in_offset=None,
                )
                for st in store_insts:
                    add_dep_helper(
                        scat.ins, st.ins, reason="scatter after bulk node stores"
                    )

            if hw_loop and loop > 1:
                # timing mode: hardware loop keeps the body IRAM-resident
                # at any repetition count
                with tc.For_i(0, loop, 1):
                    emit_body()
            else:
                for _ in range(loop):
                    emit_body()

    nc.compile()
    return nc


def _get_nc():
    if "nc" not in _CACHE:
        _CACHE["nc"] = _build()
    return _CACHE["nc"]


def _prepare(x, nodes, adj, num_nodes, W):
    """Host-side prep: shard, pre-apply the rare wrap shift, build aux
    index/correction tensors.  Returns (in_maps, num_out)."""
    x = np.asarray(x)
    nodes = np.asarray(nodes)
    adj = np.asarray(adj)
    num_nodes = np.asarray(num_nodes)
    W = np.asarray(W)

    wrap = (num_nodes.astype(np.int64) + 1) > N
    idx = np.where(wrap, num_nodes - 1, num_nodes).astype(np.int64)  # post-wrap slot
    num_out = (idx + 1).astype(num_nodes.dtype)

    x32 = np.ascontiguousarray(x, dtype=np.float32)
    W32 = np.ascontiguousarray(W, dtype=np.float32)

    in_maps = []
    for c in range(NCORES):
        s = c * SB
        nodes_s = nodes[s : s + SB]
        adj_s = adj[s : s + SB]
        wl = np.nonzero(wrap[s : s + SB])[0]
        if wl.size:
            nodes_s = nodes_s.copy()
            adj_s = adj_s.copy()
            for b in wl:
                nodes_s[b, :-1] = nodes[s + b, 1:]
                nodes_s[b, -1] = 0.0
                adj_s[b, :-1, :-1] = adj[s + b, 1:, 1:]
                adj_s[b, -1, :] = 0.0
                adj_s[b, :, -1] = 0.0
        nodes_s = np.ascontiguousarray(nodes_s, dtype=np.float32)
        adj_s = np.ascontiguousarray(adj_s, dtype=np.float32)

        li = idx[s : s + SB]
        # adj_w row of the written slot; vertical layout
        # rowsv[p, b*NT+t] = adj_row[p*NT + t]  (m = p*NT + t, block-major)
        rows = adj_s[np.arange(SB), li]              # [SB, N]
        rowsv = (
            rows.reshape(SB, 128, NT).transpose(1, 0, 2).reshape(128, SB * NT)
        )
        # rank-1 scatter fixup: ycorr[b] = adj_row[idx]*(x - nodes_w[idx])
        diag = rows[np.arange(SB), li]               # [SB]
        nrow = nodes_s[np.arange(SB), li]            # [SB, F]
        ycorr = diag[:, None] * (x32[s : s + SB] - nrow)
        ycorrv = (
            ycorr.reshape(SB, FT, 128).transpose(2, 0, 1).reshape(128, SB * FT)
        )
        sidx = (np.arange(SB) * N + li).astype(np.int32).reshape(SB, 1)

        in_maps.append(
            {
                "nodes_in": nodes_s.reshape(SB * N, F),
                "adj_in": adj_s,
                "x_in": x32[s : s + SB],
                "w_in": W32,
                "rowsv_in": np.ascontiguousarray(rowsv, dtype=np.float32),
                "ycorrv_in": np.ascontiguousarray(ycorrv, dtype=np.float32),
                "sidx_in": sidx,
            }
        )
    return in_maps, num_out


def _get_runner():
    """Persistent jitted shard_map callable over all 8 cores (mirrors
    run_bass_via_pjrt, but built once and reused across kernel() calls)."""
    if "runner" in _CACHE:
        return _CACHE["runner"]

    import warnings

    import jax
    import jax.numpy as jnp
    from jax.sharding import Mesh, NamedSharding, PartitionSpec

    try:
        with warnings.catch_warnings():
            warnings.simplefilter("ignore")
            from jax.experimental.shard_map import shard_map as _sm

        def shard_map(f, mesh, in_specs, out_specs, check_rep):
            return _sm(
                f, mesh=mesh, in_specs=in_specs, out_specs=out_specs,
                check_rep=check_rep,
            )
    except ImportError:
        from jax import shard_map as _sm

        def shard_map(f, mesh, in_specs, out_specs, check_rep):
            return _sm(f, mesh=mesh, in_specs=in_specs, out_specs=out_specs)

    import concourse.mybir as mybir
    from concourse import bass2jax
    from concourse.bass2jax import _bass_exec_p, install_neuronx_cc_hook

    nc = _get_nc()
    install_neuronx_cc_hook()
    partition_name = nc.partition_id_tensor.name if nc.partition_id_tensor else None
    in_names, out_names, out_avals = [], [], []
    for alloc in nc.m.functions[0].allocations:
        if not isinstance(alloc, mybir.MemoryLocationSet):
            continue
        name = alloc.memorylocations[0].name
        if alloc.kind == "ExternalInput":
            if name != partition_name:
                in_names.append(name)
        elif alloc.kind == "ExternalOutput":
            out_names.append(name)
            out_avals.append(
                jax.core.ShapedArray(
                    tuple(alloc.tensor_shape), mybir.dt.np(alloc.dtype)
                )
            )
    n_params = len(in_names)
    bind_names = tuple(
        in_names + out_names + ([partition_name] if partition_name else [])
    )

    def _body(*args):
        operands = list(args)
        if partition_name is not None:
            operands.append(bass2jax.partition_id_tensor())
        return tuple(
            _bass_exec_p.bind(
                *operands,
                out_avals=tuple(out_avals),
                in_names=bind_names,
                out_names=tuple(out_names),
                lowering_input_output_aliases=(),
                sim_require_finite=True,
                sim_require_nnan=True,
                nc=nc,
            )
        )

    devices = jax.devices()[:NCORES]
    mesh = Mesh(np.asarray(devices), ("core",))
    nin = n_params + len(out_names)
    donate = tuple(range(n_params, nin))
    sharded = jax.jit(
        shard_map(
            _body,
            mesh=mesh,
            in_specs=(PartitionSpec("core"),) * nin,
            out_specs=(PartitionSpec("core"),) * len(out_names),
            check_rep=False,
        ),
        donate_argnums=donate,
    )
    shard = NamedSharding(mesh, PartitionSpec("core"))

    def run(in_maps):
        dev_in = [
            jax.device_put(
                np.concatenate([np.asarray(m[nm]) for m in in_maps], axis=0),
                shard,
            )
            for nm in in_names
        ]
        zeros = [
            jax.device_put(
                jnp.zeros((NCORES * a.shape[0], *a.shape[1:]), a.dtype), shard
            )
            for a in out_avals
        ]
        outs = sharded(*dev_in, *zeros)
        return {nm: np.asarray(o) for nm, o in zip(out_names, outs)}

    _CACHE["runner"] = run
    return run


def kernel(x, nodes, adj, num_nodes, W):
    in_maps, num_out = _prepare(x, nodes, adj, num_nodes, W)

    try:
        out = _get_runner()(in_maps)
        mx = out["mx_out"]
        nodes_full = out["nodes_out"].reshape(B, N, F)
        adj_full = out["adj_out"].reshape(B, N, N)
    except Exception:
        # robust fallback: the stock SPMD path
        from concourse.bass_utils import run_bass_kernel_spmd

        _CACHE.pop("runner", None)
        res = run_bass_kernel_spmd(
            _get_nc(), in_maps, list(range(NCORES)), trace=TRACE
        )
        _CACHE["last_res"] = res
        mx = np.concatenate([r["mx_out"] for r in res.results], axis=0)
        nodes_full = np.concatenate(
            [r["nodes_out"].reshape(SB, N, F) for r in res.results], axis=0
        )
        adj_full = np.concatenate([r["adj_out"] for r in res.results], axis=0)
    return mx, nodes_full, adj_full, num_out


# revision 20
# speedup vs baseline: 3.7604x; 1.3493x over previous
"""DenseGCM kernel for 8 Trainium2 NeuronCores.

Reference semantics (per batch b of 64):
  1. wrap-eviction if num_nodes[b] == 1024 (zero slot 0, roll up by one)
  2. scatter x[b] into nodes[b, idx] (idx = post-wrap num_nodes)
  3. node_feats = tanh((adj @ nodes) @ W); mx = node_feats[idx]
  4. return (mx, nodes, adj, num_nodes + 1)

node_feats is only read back at row idx, so the GNN collapses to a
matvec chain: mx[b] = tanh((adj_w[b, idx] @ nodes_w[b]) @ W).  The
dominant cost is pure memory movement of adj (4MB/batch in + out).

Sharding: pure data parallel, 8 batches per core.  Per core:
  - adj copied DRAM->DRAM (the wrap shift for the ~1/1025-rare wrapped
    batches is applied on host when building the shard).
  - nodes streamed DRAM->SBUF->DRAM; while each [1024, 256] batch tile
    is resident, TensorE computes yT = nodes_w^T @ adj_row (vertical
    [256, 1]), then z = yT^T @ W, then ScalarE tanh -> mx row.
  - x rows scattered into nodes_out by one indirect DMA (indices are
    host-computed from num_nodes, which is an input and host-visible).
  - the scatter also perturbs y; the rank-1 fixup
    ycorr = adj_row[idx] * (x - nodes_w[idx]) is a host-computed
    256-vector added to yT on device.

Node index m maps to (partition, slot) as m = p*8 + t so each DMA
descriptor moves a contiguous 2KB run per partition.
"""

import numpy as np

B, N, F = 64, 1024, 256
NCORES = 8
SB = B // NCORES  # batches per core
NT = N // 128     # m-slots per partition
FT = F // 128     # feature half-tiles

_CACHE = {}
TRACE = False  # dev harness hook (run_bass_kernel_spmd fallback path only)


def _build(loop=1, hw_loop=False):
    import contextlib

    import concourse.mybir as mybir
    import concourse.tile as tile
    from concourse import bacc
    from concourse.bass import IndirectOffsetOnAxis
    from concourse.tile import add_dep_helper

    f32 = mybir.dt.float32
    i32 = mybir.dt.int32

    nc = bacc.Bacc("TRN2", target_bir_lowering=False, debug=False)

    nodes_in = nc.dram_tensor("nodes_in", [SB * N, F], f32, kind="ExternalInput")
    adj_in = nc.dram_tensor("adj_in", [SB, N, N], f32, kind="ExternalInput")
    x_in = nc.dram_tensor("x_in", [SB, F], f32, kind="ExternalInput")
    w_in = nc.dram_tensor("w_in", [F, F], f32, kind="ExternalInput")
    rowsv_in = nc.dram_tensor("rowsv_in", [128, SB * NT], f32, kind="ExternalInput")
    ycorrv_in = nc.dram_tensor("ycorrv_in", [128, SB * FT], f32, kind="ExternalInput")
    sidx_in = nc.dram_tensor("sidx_in", [SB, 1], i32, kind="ExternalInput")

    nodes_out = nc.dram_tensor("nodes_out", [SB * N, F], f32, kind="ExternalOutput")
    adj_out = nc.dram_tensor("adj_out", [SB, N, N], f32, kind="ExternalOutput")
    mx_out = nc.dram_tensor("mx_out", [SB, F], f32, kind="ExternalOutput")

    with tile.TileContext(nc) as tc:
        with (
            tc.tile_pool(name="const", bufs=1) as const_pool,
            tc.tile_pool(name="nodes", bufs=2) as node_pool,
            tc.tile_pool(name="small", bufs=2) as small_pool,
            tc.tile_pool(name="mx", bufs=2) as mx_pool,
            tc.tile_pool(name="yps", bufs=2, space="PSUM") as ypool,
            tc.tile_pool(name="zps", bufs=2, space="PSUM") as zpool,
        ):
            wv = const_pool.tile([128, FT, F], f32)
            nc.scalar.dma_start(wv[:], w_in[:].rearrange("(k p) g -> p k g", p=128))
            rowsv = const_pool.tile([128, SB * NT], f32)
            nc.scalar.dma_start(rowsv[:], rowsv_in[:])
            ycorrv = const_pool.tile([128, SB * FT], f32)
            nc.scalar.dma_start(ycorrv[:], ycorrv_in[:])
            xrow = const_pool.tile([SB, F], f32)
            nc.scalar.dma_start(xrow[:], x_in[:])
            sidx = const_pool.tile([SB, 1], i32)
            nc.scalar.dma_start(sidx[:], sidx_in[:])

            def emit_body():
                # adj passthrough: DRAM->DRAM, one 4MB chunk per batch on
                # the SP HWDGE ring so it streams concurrently with the
                # nodes traffic on the ACT ring.
                for b in range(SB):
                    nc.sync.dma_start(adj_out[b, :, :], adj_in[b, :, :])

                # nodes: ONE 8MB load / store for all 8 batches (DMA
                # efficiency ~97% at 8MB vs ~78% at 1MB); row index maps
                # as row = b*N + p*NT + t
                nt = node_pool.tile([128, SB, NT, F], f32)
                src = nodes_in[:].rearrange("(b p t) f -> p b t f", p=128, t=NT)
                nc.scalar.dma_start(nt[:], src)

                store_insts = []
                for b in range(SB):
                    # yT[fh*128+p] = sum_m nodes_w[m, f] * adj_row[m]
                    yps = ypool.tile([128, FT], f32)
                    for fh in range(FT):
                        for t in range(NT):
                            nc.tensor.matmul(
                                yps[:, fh : fh + 1],
                                lhsT=nt[:, b, t, fh * 128 : (fh + 1) * 128],
                                rhs=rowsv[:, b * NT + t : b * NT + t + 1],
                                start=(t == 0),
                                stop=(t == NT - 1),
                            )
                    yt = small_pool.tile([128, FT], f32)
                    nc.vector.tensor_add(
                        yt[:], yps[:], ycorrv[:, b * FT : (b + 1) * FT]
                    )

                    # z[g] = sum_f yT[f] * W[f, g]
                    zps = zpool.tile([1, F], f32)
                    for kt in range(FT):
                        nc.tensor.matmul(
                            zps[:1, :],
                            lhsT=yt[:, kt : kt + 1],
                            rhs=wv[:, kt, :],
                            start=(kt == 0),
                            stop=(kt == FT - 1),
                        )
                    mxs = mx_pool.tile([1, F], f32)
                    nc.scalar.activation(
                        mxs[:1, :], zps[:1, :], mybir.ActivationFunctionType.Tanh
                    )
                    nc.scalar.dma_start(mx_out[b : b + 1, :], mxs[:1, :])

                dst = nodes_out[:].rearrange("(b p t) f -> p b t f", p=128, t=NT)
                store_insts.append(nc.scalar.dma_start(dst, nt[:]))

                # scatter x rows into nodes_out after the bulk stores
                scat = nc.gpsimd.indirect_dma_start(
                    out=nodes_out[:],
                    out_offset=IndirectOffsetOnAxis(ap=sidx[:, :1], axis=0),
                    in_=xrow[:],
                    in_offset=None,
                )
                for st in store_insts:
                    add_dep_helper(
                        scat.ins, st.ins, reason="scatter after bulk node stores"
                    )

            if hw_loop and loop > 1:
                # timing mode: hardware loop keeps the body IRAM-resident
                # at any repetition count
                with tc.For_i(0, loop, 1):
                    emit_body()
            else:
                for _ in range(loop):
                    emit_body()

    nc.compile()
    return nc


def _get_nc():
    if "nc" not in _CACHE:
        _CACHE["nc"] = _build()
    return _CACHE["nc"]


def _prepare(x, nodes, adj, num_nodes, W):
    """Host-side prep: shard, pre-apply the rare wrap shift, build aux
    index/correction tensors.  Returns (in_maps, num_out)."""
    x = np.asarray(x)
    nodes = np.asarray(nodes)
    adj = np.asarray(adj)
    num_nodes = np.asarray(num_nodes)
    W = np.asarray(W)

    wrap = (num_nodes.astype(np.int64) + 1) > N
    idx = np.where(wrap, num_nodes - 1, num_nodes).astype(np.int64)  # post-wrap slot
    num_out = (idx + 1).astype(num_nodes.dtype)

    x32 = np.ascontiguousarray(x, dtype=np.float32)
    W32 = np.ascontiguousarray(W, dtype=np.float32)

    in_maps = []
    for c in range(NCORES):
        s = c * SB
        nodes_s = nodes[s : s + SB]
        adj_s = adj[s : s + SB]
        wl = np.nonzero(wrap[s : s + SB])[0]
        if wl.size:
            nodes_s = nodes_s.copy()
            adj_s = adj_s.copy()
            for b in wl:
                nodes_s[b, :-1] = nodes[s + b, 1:]
                nodes_s[b, -1] = 0.0
                adj_s[b, :-1, :-1] = adj[s + b, 1:, 1:]
                adj_s[b, -1, :] = 0.0
                adj_s[b, :, -1] = 0.0
        nodes_s = np.ascontiguousarray(nodes_s, dtype=np.float32)
        adj_s = np.ascontiguousarray(adj_s, dtype=np.float32)

        li = idx[s : s + SB]
        # adj_w row of the written slot; vertical layout
        # rowsv[p, b*NT+t] = adj_row[p*NT + t]  (m = p*NT + t, block-major)
        rows = adj_s[np.arange(SB), li]              # [SB, N]
        rowsv = (
            rows.reshape(SB, 128, NT).transpose(1, 0, 2).reshape(128, SB * NT)
        )
        # rank-1 scatter fixup: ycorr[b] = adj_row[idx]*(x - nodes_w[idx])
        diag = rows[np.arange(SB), li]               # [SB]
        nrow = nodes_s[np.arange(SB), li]            # [SB, F]
        ycorr = diag[:, None] * (x32[s : s + SB] - nrow)
        ycorrv = (
            ycorr.reshape(SB, FT, 128).transpose(2, 0, 1).reshape(128, SB * FT)
        )
        sidx = (np.arange(SB) * N + li).astype(np.int32).reshape(SB, 1)

        in_maps.append(
            {
                "nodes_in": nodes_s.reshape(SB * N, F),
                "adj_in": adj_s,
                "x_in": x32[s : s + SB],
                "w_in": W32,
                "rowsv_in": np.ascontiguousarray(rowsv, dtype=np.float32),
                "ycorrv_in": np.ascontiguousarray(ycorrv, dtype=np.float32),
                "sidx_in": sidx,
            }
        )
    return in_maps, num_out


def _get_runner():
    """Persistent jitted shard_map callable over all 8 cores (mirrors
    run_bass_via_pjrt, but built once and reused across kernel() calls)."""
    if "runner" in _CACHE:
        return _CACHE["runner"]

    import warnings

    import jax
    import jax.numpy as jnp
    from jax.sharding import Mesh, NamedSharding, PartitionSpec

    try:
        with warnings.catch_warnings():
            warnings.simplefilter("ignore")
            from jax.experimental.shard_map import shard_map as _sm

        def shard_map(f, mesh, in_specs, out_specs, check_rep):
            return _sm(
                f, mesh=mesh, in_specs=in_specs, out_specs=out_specs,
                check_rep=check_rep,
            )
    except ImportError:
        from jax import shard_map as _sm

        def shard_map(f, mesh, in_specs, out_specs, check_rep):
            return _sm(f, mesh=mesh, in_specs=in_specs, out_specs=out_specs)

    import concourse.mybir as mybir
    from concourse import bass2jax
    from concourse.bass2jax import _bass_exec_p, install_neuronx_cc_hook

    nc = _get_nc()
    install_neuronx_cc_hook()
    partition_name = nc.partition_id_tensor.name if nc.partition_id_tensor else None
    in_names, out_names, out_avals = [], [], []
    for alloc in nc.m.functions[0].allocations:
        if not isinstance(alloc, mybir.MemoryLocationSet):
            continue
        name = alloc.memorylocations[0].name
        if alloc.kind == "ExternalInput":
            if name != partition_name:
                in_names.append(name)
        elif alloc.kind == "ExternalOutput":
            out_names.append(name)
            out_avals.append(
                jax.core.ShapedArray(
                    tuple(alloc.tensor_shape), mybir.dt.np(alloc.dtype)
                )
            )
    n_params = len(in_names)
    bind_names = tuple(
        in_names + out_names + ([partition_name] if partition_name else [])
    )

    def _body(*args):
        operands = list(args)
        if partition_name is not None:
            operands.append(bass2jax.partition_id_tensor())
        return tuple(
            _bass_exec_p.bind(
                *operands,
                out_avals=tuple(out_avals),
                in_names=bind_names,
                out_names=tuple(out_names),
                lowering_input_output_aliases=(),
                sim_require_finite=True,
                sim_require_nnan=True,
                nc=nc,
            )
        )

    devices = jax.devices()[:NCORES]
    mesh = Mesh(np.asarray(devices), ("core",))
    nin = n_params + len(out_names)
    donate = tuple(range(n_params, nin))
    sharded = jax.jit(
        shard_map(
            _body,
            mesh=mesh,
            in_specs=(PartitionSpec("core"),) * nin,
            out_specs=(PartitionSpec("core"),) * len(out_names),
            check_rep=False,
        ),
        donate_argnums=donate,
    )
    shard = NamedSharding(mesh, PartitionSpec("core"))

    def run(in_maps):
        dev_in = [
            jax.device_put(
                np.concatenate([np.asarray(m[nm]) for m in in_maps], axis=0),
                shard,
            )
            for nm in in_names
        ]
        zeros = [
            jax.device_put(
                jnp.zeros((NCORES * a.shape[0], *a.shape[1:]), a.dtype), shard
            )
            for a in out_avals
        ]
        outs = sharded(*dev_in, *zeros)
        return {nm: np.asarray(o) for nm, o in zip(out_names, outs)}

    _CACHE["runner"] = run
    return run


def kernel(x, nodes, adj, num_nodes, W):
    in_maps, num_out = _prepare(x, nodes, adj, num_nodes, W)

    try:
        out = _get_runner()(in_maps)
        mx = out["mx_out"]
        nodes_full = out["nodes_out"].reshape(B, N, F)
        adj_full = out["adj_out"].reshape(B, N, N)
    except Exception:
        # robust fallback: the stock SPMD path
        from concourse.bass_utils import run_bass_kernel_spmd

        _CACHE.pop("runner", None)
        res = run_bass_kernel_spmd(
            _get_nc(), in_maps, list(range(NCORES)), trace=TRACE
        )
        _CACHE["last_res"] = res
        mx = np.concatenate([r["mx_out"] for r in res.results], axis=0)
        nodes_full = np.concatenate(
            [r["nodes_out"].reshape(SB, N, F) for r in res.results], axis=0
        )
        adj_full = np.concatenate([r["adj_out"] for r in res.results], axis=0)
    return mx, nodes_full, adj_full, num_out
